# revision 16
# baseline (speedup 1.0000x reference)
"""DA3CrossFrameRKDAngleLoss Trainium2 kernel (bass/Tile).  v6

Sharding: 8 cores = (batch b = core//2) x (ref-row half = core%2).
Each core handles R=128 ref rows of one batch; host sums partial sums.

v6 key idea: the gather STREAM is ordered m = k*128 + r (k-major).  Then
  - sim_hi (non-transpose gather) lands with partition = ref row r
  - simT columns m=k*128+r make P1's psum partition = r for group g=k
  - the T / q_sr "replication to rk" becomes the IDENTITY -> plain
    broadcast APs along the k axis; no replication matmuls at all
  - all per-(r,k) scalars (rr, hh, rh, qhr, ...) live in [r, k] layout
Other v6 changes: Dsqrt(0.25x)=rsqrt(x) on ACT replaces sqrt+recip+cast;
a3 numerator from the identity a3num = t6 - m2 + bsh (sign-free under
abs); dd on gpsimd; 1-MiB ext chunk DMAs alternating the two hw queues.

Per-core math (R=128, S=256, K=4, D=1024, E=4096):
  sim[r,e] = ref_t[r] . extra_unit[e]   fp16 (row scale irrelevant to topk)
  top4 -> stream m=k*128+r of extra row ids; gathers: sim_hi [r,K,D],
  simT [d%128, d//128, m]
  sr'[r,(f,s)] = ref.shared   fp16 MMs; T = -2sr' + (rr+ss);
  qsr = rsqrt(T); P1[m=(k,r),s] = sim.shared - sr'  (identity subtract MM)
  per (f,n):  t6 = T - 2 P1;  qsh = rsqrt(t6 + bsh)
    a1 = (qhr P1 + b1) qsr ;  a2 = (u2 - P1) qhr qsh
    a3 = -(T - P1 - u1) qsr qsh  (global sign free under |a_t - a_s|)
  acc[f,a] = sum |a_teacher - a_student|
loss = sum(acc) / (3*B*256*256*4)
"""
import sys
sys.path.insert(0, '/opt/trn_rl_repo')
import numpy as np

import concourse.bass as bass
import concourse.mybir as mybir
import concourse.tile as tile
from concourse import bacc
from concourse.bass_utils import run_bass_kernel_spmd

AF = mybir.ActivationFunctionType
OP = mybir.AluOpType
F32 = mybir.dt.float32
F16 = mybir.dt.float16

R, S, K, D, E = 128, 256, 4, 1024, 4096
RK = R * K
NF = 3
KC = D // 128          # 8
EC = E // 512          # 8


def build_program(n_cores=8, use_dsqrt=False):
    nc = bacc.Bacc("TRN2", target_bir_lowering=False, debug=False,
                   num_devices=n_cores, num_swdge_queues=2)
    d = {}
    d['extra_nt'] = nc.dram_tensor("extra_nt", [D, E], F16, kind="ExternalInput").ap()
    d['reft_t'] = nc.dram_tensor("reft_t", [D, R], F16, kind="ExternalInput").ap()
    d['ref_sr'] = nc.dram_tensor("ref_sr", [2, D, R], F16, kind="ExternalInput").ap()
    d['sh_f'] = nc.dram_tensor("sh_f", [NF, D, 2 * S], F16, kind="ExternalInput").ap()
    d['w_rs'] = nc.dram_tensor("w_rs", [NF, 2, 128, S], F32, kind="ExternalInput").ap()
    d['extra_g'] = nc.dram_tensor("extra_g", [E, D], F16, kind="ExternalInput").ap()
    d['ref_rows'] = nc.dram_tensor("ref_rows", [2, R, D], F16, kind="ExternalInput").ap()
    d['rr'] = nc.dram_tensor("rr", [2, 128, K], F32, kind="ExternalInput").ap()
    d['id128'] = nc.dram_tensor("id128", [128, 128], F16, kind="ExternalInput").ap()
    d['acc'] = nc.dram_tensor("acc", [128, NF * 3], F32, kind="ExternalOutput").ap()
    d['idx'] = nc.dram_tensor("idx", [128, 8], mybir.dt.uint32, kind="ExternalOutput").ap()

    with tile.TileContext(nc) as tc:
        _body(nc, tc, d, use_dsqrt)
    nc.compile()
    return nc


def _rsqrt(nc, eb, out_f16, in_ap, use_dsqrt, bias=0.0, shape=None):
    """out = 1/sqrt(in + bias).  Dsqrt(0.25x + 0.25b) on HW; sim fallback."""
    if use_dsqrt:
        nc.scalar.activation(out_f16, in_ap, AF.Dsqrt, scale=0.25, bias=bias)
    else:
        shp = shape or list(out_f16.shape)
        t1 = eb.tile(shp, F32, tag="rsq1", name="rsq1")
        nc.scalar.activation(t1[:], in_ap, AF.Sqrt,
                             bias=(bias if not isinstance(bias, float) else bias))
        t2 = eb.tile(shp, F32, tag="rsq2", name="rsq2")
        nc.vector.reciprocal_approx_fast(out=t2[:], in_=t1[:])
        nc.vector.tensor_copy(out=out_f16, in_=t2[:])


def _body(nc, tc, d, use_dsqrt):
    from contextlib import ExitStack
    with ExitStack() as ctx:
        sb = ctx.enter_context(tc.tile_pool(name="persist", bufs=1))

        # ---- resident tiles ----
        sh = [sb.tile([128, KC, 2 * S], F16, tag=f"sh{f}", name=f"sh{f}") for f in range(NF)]
        w_rs = [[sb.tile([128, S], F32, tag=f"w{f}{n}", name=f"w{f}{n}")
                 for n in range(2)] for f in range(NF)]
        ref_rows = [sb.tile([128, D], F16, tag=f"refr{n}", name=f"refr{n}") for n in range(2)]
        rr = [sb.tile([128, K], F32, tag=f"rr{n}", name=f"rr{n}") for n in range(2)]
        id128 = sb.tile([128, 128], F16, tag="id128", name="id128")
        sim_hi = sb.tile([128, K, D], F16, tag="sim_hi", name="sim_hi")
        simT = sb.tile([128, KC, RK], F16, tag="simT", name="simT")
        T_sb = [[sb.tile([128, S], F16, tag=f"T{f}{n}", name=f"T{f}{n}")
                 for n in range(2)] for f in range(NF)]
        qsr_sb = [[sb.tile([128, S], F16, tag=f"qsr{f}{n}", name=f"qsr{f}{n}")
                   for n in range(2)] for f in range(NF)]
        nsrp = [sb.tile([128, 2, S], F16, tag=f"nsrp{f}", name=f"nsrp{f}") for f in range(NF)]
        acc = sb.tile([128, NF * 3], F32, tag="acc", name="acc")
        refb = [sb.tile([128, KC, R], F16, tag=f"refb{n}", name=f"refb{n}") for n in range(2)]

        with tc.tile_pool(name="early", bufs=1) as eb:
            reft = eb.tile([128, KC, R], F16, tag="reft", name="reft")
            nc.sync.dma_start(reft[:], d['reft_t'].rearrange("(c p) r -> p c r", p=128))
            sim_sb = eb.tile([128, E], F16, tag="sim_sb", name="sim_sb")
            cmx = eb.tile([128, K, 8], F16, tag="cmx", name="cmx")

            # ---- phase 1: sim, kc-outer, full-PSUM accumulation ----
            # 1-MiB ext chunks alternate the two hwdge queues; bulk loads
            # are emitted AFTER the ext chunks on each queue.
            with tc.tile_pool(name="ext", bufs=3) as extp, \
                 tc.tile_pool(name="simps", bufs=1, space="PSUM") as simps:
                sim_ps = simps.tile([128, E], F32, tag="sim_ps", name="sim_ps")
                for kc in range(KC):
                    x = extp.tile([128, E], F16, tag="ext", name="ext")
                    eng = (nc.sync, nc.scalar)[kc % 2]
                    eng.dma_start(x[:], d['extra_nt'][kc * 128:(kc + 1) * 128, :])
                    last = kc == KC - 1
                    for e in range(EC):
                        nc.tensor.matmul(sim_ps[:, e * 512:(e + 1) * 512],
                                         reft[:, kc, :], x[:, e * 512:(e + 1) * 512],
                                         start=(kc == 0), stop=last)
                        if last:
                            if e % 2 == 0:
                                nc.scalar.copy(sim_sb[:, e * 512:(e + 1) * 512],
                                               sim_ps[:, e * 512:(e + 1) * 512])
                            else:
                                nc.vector.tensor_copy(out=sim_sb[:, e * 512:(e + 1) * 512],
                                                      in_=sim_ps[:, e * 512:(e + 1) * 512])
                                nc.vector.max(out=cmx[:, e // 2, :],
                                              in_=sim_sb[:, (e - 1) * 512:(e + 1) * 512])
                # bulk loads, behind the ext chunks on each queue
                for n in range(2):
                    nc.scalar.dma_start(refb[n][:],
                                        d['ref_sr'][n].rearrange("(c p) r -> p c r", p=128))
                for f in range(NF):
                    nc.scalar.dma_start(sh[f][:],
                                        d['sh_f'][f].rearrange("(c p) s -> p c s", p=128))
                nc.sync.dma_start(id128[:], d['id128'])
                for f in range(NF):
                    for n in range(2):
                        nc.sync.dma_start(w_rs[f][n][:], d['w_rs'][f, n])
                for n in range(2):
                    nc.sync.dma_start(rr[n][:], d['rr'][n])
                    nc.sync.dma_start(ref_rows[n][:], d['ref_rows'][n])

            # ---- phase 1b: sr' per net (PE fills while DVE finishes topk) ----
            sp3 = [None] * 2
            with tc.tile_pool(name="srps", bufs=2, space="PSUM") as srps:
                for n in range(2):
                    sp3[n] = srps.tile([128, NF, S], F32, tag="sp3", name=f"sp3_{n}")
                    for f in range(NF):
                        for kc in range(KC):
                            nc.tensor.matmul(sp3[n][:, f, :], refb[n][:, kc, :],
                                             sh[f][:, kc, n * S:(n + 1) * S],
                                             start=(kc == 0), stop=(kc == KC - 1))

                # ---- phase 2: topk + gathers (stream order m = k*128 + r) ----
                mx = eb.tile([128, 8], F16, tag="mx", name="mx")
                mi = eb.tile([128, 8], mybir.dt.uint32, tag="mi", name="mi")
                nc.vector.max(out=mx[:], in_=cmx[:].rearrange("p a b -> p (a b)"))
                nc.vector.max_index(out=mi[:], in_max=mx[:], in_values=sim_sb[:])
                nc.sync.dma_start(d['idx'][:], mi[:])
                idx16 = eb.tile([128, K], mybir.dt.int16, tag="idx16", name="idx16")
                nc.vector.tensor_copy(out=idx16[:], in_=mi[:, 0:K])

                with tc.tile_pool(name="dram", bufs=1, space="DRAM") as drp:
                    idx_dram = drp.tile([RK], mybir.dt.int16, name="idx_dram")
                    # k-major stream: idx_dram[k*128 + r] = idx16[r, k]
                    nc.sync.dma_start(idx_dram[:].rearrange("(a p) -> p a", p=128),
                                      idx16[:])
                    idxw = eb.tile([128, RK // 16], mybir.dt.int16, tag="idxw", name="idxw")
                    wrapped = idx_dram[:].rearrange("(j q) -> q j", q=16)
                    for sg in range(8):
                        eng = (nc.sync, nc.scalar)[sg % 2]
                        eng.dma_start(idxw[16 * sg:16 * (sg + 1), :], wrapped)
                    nc.gpsimd.dma_gather(simT[:], d['extra_g'], idxw[:], RK, RK, D,
                                         transpose=True, queue_num=0)
                    nc.gpsimd.dma_gather(sim_hi[:], d['extra_g'], idxw[:], RK, RK, D,
                                         queue_num=1)

                # ---- phase 2b: T fold, q_sr, nsrp (from sp3 psum) ----
                for n in range(2):
                    for f in range(NF):
                        nc.vector.scalar_tensor_tensor(out=T_sb[f][n][:],
                                                       in0=sp3[n][:, f, :], scalar=-2.0,
                                                       in1=w_rs[f][n][:],
                                                       op0=OP.mult, op1=OP.add)
                        _rsqrt(nc, eb, qsr_sb[f][n][:], T_sb[f][n][:], use_dsqrt,
                               shape=[128, S])
                        nc.scalar.activation(nsrp[f][:, n, :], sp3[n][:, f, :],
                                             AF.Copy, scale=-1.0)

        # ---- phase 4: per-(r,k) scalars (needs sim_hi; [r,k] layout) ----
        hh = sb.tile([128, K], F32, tag="hh", name="hh")
        dump = sb.tile([128, D], F16, tag="dump", name="dump")
        for g in range(K):
            nc.vector.scalar_tensor_tensor(out=dump[:], in0=sim_hi[:, g, :],
                                           scalar=0.0, in1=sim_hi[:, g, :],
                                           op0=OP.bypass, op1=OP.mult,
                                           accum_out=hh[:, g:g + 1])
        rh = [sb.tile([128, K], F32, tag=f"rh{n}", name=f"rh{n}") for n in range(2)]
        for n in range(2):
            for g in range(K):
                nc.vector.scalar_tensor_tensor(out=dump[:], in0=sim_hi[:, g, :],
                                               scalar=0.0, in1=ref_rows[n][:],
                                               op0=OP.bypass, op1=OP.mult,
                                               accum_out=rh[n][:, g:g + 1])
        u2 = [None] * 2; b1 = [None] * 2; bsh = [None] * 2; bsh4 = [None] * 2
        qhr = [None] * 2
        with tc.tile_pool(name="ph4", bufs=1) as p4:
            for n in range(2):
                u1 = p4.tile([128, K], F32, tag=f"u1{n}", name=f"u1{n}")
                u2[n] = sb.tile([128, K], F32, tag=f"u2{n}", name=f"u2{n}")
                nc.vector.tensor_sub(out=u1[:], in0=rr[n][:], in1=rh[n][:])
                nc.vector.tensor_sub(out=u2[n][:], in0=hh[:], in1=rh[n][:])
                nhr = p4.tile([128, K], F32, tag=f"nhr{n}", name=f"nhr{n}")
                nc.vector.tensor_add(out=nhr[:], in0=u1[:], in1=u2[n][:])
                qhr[n] = sb.tile([128, K], F32, tag=f"qhr{n}", name=f"qhr{n}")
                if use_dsqrt:
                    nc.scalar.activation(qhr[n][:], nhr[:], AF.Dsqrt, scale=0.25)
                else:
                    nc.scalar.activation(nhr[:], nhr[:], AF.Sqrt, bias=0.0)
                    nc.vector.reciprocal_approx_fast(out=qhr[n][:], in_=nhr[:])
                b1[n] = sb.tile([128, K], F32, tag=f"b1{n}", name=f"b1{n}")
                nc.vector.tensor_mul(out=b1[n][:], in0=u1[:], in1=qhr[n][:])
                bsh[n] = sb.tile([128, K], F32, tag=f"bsh{n}", name=f"bsh{n}")
                nc.vector.tensor_sub(out=bsh[n][:], in0=u2[n][:], in1=u1[:])
                bsh4[n] = sb.tile([128, K], F32, tag=f"bsh4{n}", name=f"bsh4{n}")
                nc.vector.tensor_scalar_mul(bsh4[n][:], bsh[n][:], 0.25)

        # ---- phases 5-6: P1 + angles, pipelined per frame ----
        with tc.tile_pool(name="p1ps", bufs=2, space="PSUM") as p1p, \
             tc.tile_pool(name="ang", bufs=2) as ang, \
             tc.tile_pool(name="aout", bufs=1) as aoutp, \
             tc.tile_pool(name="angb", bufs=2) as angb, \
             tc.tile_pool(name="rsqp", bufs=2) as rsqp:
            for f in range(NF):
                a_out = [[None] * 2 for _ in range(3)]
                p1f = p1p.tile([128, K, 2 * S], F32, tag="p1", name="p1")
                for g in range(K):
                    nc.tensor.matmul(p1f[:, g, :], id128[:],
                                     nsrp[f][:].rearrange("p a b -> p (a b)"),
                                     start=True, stop=False)
                for kc in range(KC):
                    for g in range(K):
                        nc.tensor.matmul(p1f[:, g, :],
                                         simT[:, kc, g * 128:(g + 1) * 128],
                                         sh[f][:, kc, :],
                                         start=False, stop=(kc == KC - 1))
                for n in range(2):
                    p1v = p1f[:, :, n * S:(n + 1) * S]        # [128, K, S] psum view
                    T_bc = T_sb[f][n][:].unsqueeze(1).broadcast_to([128, K, S])
                    q_bc = qsr_sb[f][n][:].unsqueeze(1).broadcast_to([128, K, S])
                    ta1 = ang.tile([128, K, S], F16, tag="ta1", name="ta1")
                    m2 = ang.tile([128, K, S], F16, tag="m2", name="m2")
                    for g in range(K):
                        nc.scalar.activation(ta1[:, g, :], p1f[:, g, n * S:(n + 1) * S],
                                             AF.Identity, scale=qhr[n][:, g:g + 1],
                                             bias=b1[n][:, g:g + 1])
                        nc.scalar.activation(m2[:, g, :], p1f[:, g, n * S:(n + 1) * S],
                                             AF.Identity, scale=-1.0,
                                             bias=u2[n][:, g:g + 1])
                    t6 = ang.tile([128, K, S], F16, tag="t6", name="t6")
                    nc.vector.scalar_tensor_tensor(out=t6[:], in0=p1v, scalar=-2.0,
                                                   in1=T_bc, op0=OP.mult, op1=OP.add)
                    qsh = ang.tile([128, K, S], F16, tag="qsh", name="qsh")
                    if use_dsqrt:
                        for g in range(K):
                            nc.scalar.activation(qsh[:, g, :], t6[:, g, :], AF.Dsqrt,
                                                 scale=0.25, bias=bsh4[n][:, g:g + 1])
                    else:
                        nshf = rsqp.tile([128, K, S], F32, tag="nshf", name="nshf")
                        for g in range(K):
                            nc.scalar.activation(nshf[:, g, :], t6[:, g, :], AF.Sqrt,
                                                 bias=bsh[n][:, g:g + 1])
                        qshf = rsqp.tile([128, K, S], F32, tag="qshf", name="qshf")
                        nc.vector.reciprocal_approx_fast(out=qshf[:], in_=nshf[:])
                        nc.vector.tensor_copy(out=qsh[:], in_=qshf[:])
                    a1 = aoutp.tile([128, K, S], F16, tag=f"a1_{n}", name=f"a1_{n}")
                    a2 = aoutp.tile([128, K, S], F16, tag=f"a2_{n}", name=f"a2_{n}")
                    a3 = aoutp.tile([128, K, S], F16, tag=f"a3_{n}", name=f"a3_{n}")
                    nc.vector.tensor_mul(out=a1[:], in0=ta1[:], in1=q_bc)
                    for g in range(K):
                        nc.vector.scalar_tensor_tensor(out=a2[:, g, :], in0=m2[:, g, :],
                                                       scalar=qhr[n][:, g:g + 1],
                                                       in1=qsh[:, g, :],
                                                       op0=OP.mult, op1=OP.mult)
                    a3n = ang.tile([128, K, S], F16, tag="a3n", name="a3n")
                    for g in range(K):
                        nc.vector.scalar_tensor_tensor(out=a3n[:, g, :], in0=m2[:, g, :],
                                                       scalar=bsh[n][:, g:g + 1],
                                                       op0=OP.subtract,
                                                       in1=t6[:, g, :], op1=OP.subtract)
                    a3q = ang.tile([128, K, S], F16, tag="a3q", name="a3q")
                    nc.vector.tensor_mul(out=a3q[:], in0=a3n[:], in1=q_bc)
                    nc.vector.tensor_mul(out=a3[:], in0=a3q[:], in1=qsh[:])
                    a_out[0][n], a_out[1][n], a_out[2][n] = a1, a2, a3
                for a in range(3):
                    dd = angb.tile([128, K, S], F16, tag="dd", name="dd")
                    nc.vector.tensor_sub(out=dd[:], in0=a_out[a][0][:],
                                         in1=a_out[a][1][:])
                    ddump = angb.tile([128, K, S], F16, tag="ddump", name="ddump")
                    nc.vector.scalar_tensor_tensor(out=ddump[:], in0=dd[:], scalar=-1.0,
                                                   in1=dd[:], op0=OP.mult, op1=OP.max,
                                                   accum_out=acc[:, 3 * f + a:3 * f + a + 1])
        nc.sync.dma_start(d['acc'][:], acc[:])


# ======================= host side =======================

EXTRA_FRAMES = (1, 3, 5, 7)
SHARED_TEACHER = (2, 4, 6)
SHARED_STUDENT = (1, 2, 3)


def host_prep(teacher_feats, student_feats, ref_perm, shared_perm, n_cores=8):
    t = np.ascontiguousarray(np.asarray(teacher_feats, dtype=np.float32))
    s = np.ascontiguousarray(np.asarray(student_feats, dtype=np.float32))
    rp = np.asarray(ref_perm).astype(np.int64)
    sp = np.asarray(shared_perm).astype(np.int64)
    id128 = np.eye(128, dtype=np.float16)
    in_maps = []
    for c in range(n_cores):
        b, half = c // 2, c % 2
        rows = rp[half * R:(half + 1) * R]
        ref2 = np.stack([t[b, 0][rows], s[b, 0][rows]])           # [2, R, D]
        extra = t[b][list(EXTRA_FRAMES)].reshape(E, D)
        en = extra / np.maximum(np.linalg.norm(extra, axis=1, keepdims=True), 1e-12)
        reftr = np.stack([np.ascontiguousarray(ref2[0].T),
                          np.ascontiguousarray(ref2[1].T)])       # [2, D, R]
        rrv = (ref2.astype(np.float64) ** 2).sum(-1).astype(np.float32)
        sh_f = np.zeros((NF, D, 2 * S), dtype=np.float16)
        w_rs = np.zeros((NF, 2, 128, S), dtype=np.float32)
        for f in range(NF):
            sh_t = t[b, SHARED_TEACHER[f]][sp]
            sh_s = s[b, SHARED_STUDENT[f]][sp]
            sh_f[f, :, :S] = sh_t.T
            sh_f[f, :, S:] = sh_s.T
            ss_t = (sh_t.astype(np.float64) ** 2).sum(-1)
            ss_s = (sh_s.astype(np.float64) ** 2).sum(-1)
            w_rs[f, 0] = rrv[0][:, None] + ss_t[None, :]
            w_rs[f, 1] = rrv[1][:, None] + ss_s[None, :]
        rr_rk = np.broadcast_to(rrv[:, :, None], (2, 128, K))
        in_maps.append(dict(
            extra_nt=np.ascontiguousarray(en.T).astype(np.float16),
            reft_t=reftr[0].astype(np.float16),
            ref_sr=reftr.astype(np.float16),
            sh_f=sh_f,
            w_rs=w_rs,
            extra_g=extra.astype(np.float16),
            ref_rows=ref2.astype(np.float16),
            rr=np.ascontiguousarray(rr_rk).astype(np.float32),
            id128=id128,
        ))
    return in_maps


def host_finish(results, B=4):
    total = 0.0
    for r in results:
        total += float(np.asarray(r["acc"], dtype=np.float64).sum())
    denom = NF * B * 256 * S * K
    return np.array(total / denom, dtype=np.float32)


# ======================= self-contained entry =======================

_NC_CACHE = {}


def kernel(teacher_feats, student_feats, ref_perm, shared_perm):
    """Full-input entry: shards across 8 NeuronCores, returns scalar loss (np.float32)."""
    inputs = dict(teacher_feats=np.asarray(teacher_feats),
                  student_feats=np.asarray(student_feats),
                  ref_perm=np.asarray(ref_perm),
                  shared_perm=np.asarray(shared_perm))
    n_cores = 8
    if 'nc' not in _NC_CACHE:
        _NC_CACHE['nc'] = build_program(n_cores=n_cores)
    nc = _NC_CACHE['nc']
    in_maps = host_prep(**inputs, n_cores=n_cores)
    res = run_bass_kernel_spmd(nc, in_maps, core_ids=list(range(n_cores)))
    return host_finish(res.results, B=int(inputs['teacher_feats'].shape[0]))
